# revision 39
# baseline (speedup 1.0000x reference)
"""Trainium2 Bass kernel for nn_Attention_54142357733562 (linear/sparse attention).

Reference math (per batch b, with x flattened to [C, N]):
    Q = wq @ x ; K = wk @ x ; V = wv @ x            (1x1 convs, + zero biases)
    Qn = Q / ||Q||_c ; Kn = K / ||K||_c             (L2 norm over channel dim)
    k_sum = sum_n Kn + EPS                          [Cqk]
    tailor = 1 / (N + Qn^T k_sum)                   [N]
    kv = Kn V^T                                     [Cqk, C]
    out = (value_sum + kv^T Qn) * tailor            [C, N]

Algebraic reformulation used here (avoids materializing Qn / tailor):
    s[n]   = ||Q[:, n]||
    den[n] = N*s[n] + Q[:, n]. k_sum
    out[c,n] = (U[c,n] + value_sum[c]*s[n]) / den[n],   U = kv^T Q
which is computed as a single matmul with the scale folded into the rhs:
    Q''[m,n] = [Q; s][m,n] / den[n]      (per-n scale, applied in [n,m] layout)
    out[c,n] = sum_m [kv; value_sum][m,c] * Q''[m,n]

Sharding: 8 cores = 4 batches x 2 N-halves. Phase 1 computes per-shard
partial (kv | k_sum | value_sum) = [Kn|1]^T [V|1]; an AllReduce over the
2-core pair completes the N reduction; phase 2 computes outputs for the
shard's N range. All I/O and matmul operands are bf16 (fro tolerance 2e-2);
accumulations (kv, den) stay f32.

Engine layout per macro-tile (NT=1024 = 8 sub-tiles of 128 n):
  PE   : 16 QKV matmuls, 8 kv-accumulate matmuls (skewed one macro so the
         normalize chain never stalls the in-order PE stream), 4 paired
         transposes + 4 out matmuls in phase 2 (same skew).
  DVE  : psum->stash QK copies (bf16 2x), half the V copies, reduces,
         reciprocals, half the psum drains in phase 2.
  ACT  : sqrt, other half of V copies / psum drains.
  POOL : squares, Kn normalize, ones fills, den products, qsc scale.
  SP   : one DMA per macro per direction (HWDGE dispatch is ~625ns each).
"""

import os
import numpy as np
import ml_dtypes
from contextlib import ExitStack

import concourse.bass as bass
import concourse.mybir as mybir
import concourse.tile as tile
from concourse import bacc
from concourse.bass_utils import run_bass_kernel_spmd
from concourse.masks import make_identity

F32 = mybir.dt.float32
F32R = mybir.dt.float32r
BF16 = mybir.dt.bfloat16


def _mdt(mm_dtype):
    return {"f32r": F32R, "f32": F32, "bf16": BF16}[mm_dtype]


def _np_io(mm_dtype):
    return ml_dtypes.bfloat16 if mm_dtype == "bf16" else np.float32


C = 256
CQK = 32
J = 2 * CQK + C  # 320 = stacked [Q|K|V] output channels
EPS = 1e-6
P = 128
NT = 1024  # macro-tile width along N
ST = NT // P  # 8 sub-tiles per macro
HS = 2  # sub-tiles per qkv psum tile (1 bf16 bank)

# stash row layout (W=68): [Q 0:32 | s 32 | ||K|| 33 | K 34:66 | pad]
# ([Q|s] contiguous at 0:33 is what phase 2 consumes)
SW = 68
# kvt row layout (W=292): [Kn 0:32 | 1 at 32 | pad | V 34:290 | 1 1 290:292]
KW = 292


def emit_attention(tc, xs, wt, out, nsh, n_total, groups, mm_dtype="bf16",
                   use_collective=True, phases=(1, 2)):
    """Emit the per-core SPMD program.

    xs : DRAM [C, nsh]  per-core shard of x (C-major)
    wt : DRAM [C, J]    stacked transposed weights [wq.T | wk.T | wv.T]
    out: DRAM [C, nsh]  per-core shard of the output
    """
    nc = tc.nc
    NM = nsh // NT
    SROW = nsh // P

    MDT = _mdt(mm_dtype)
    ODT = BF16 if mm_dtype == "bf16" else F32
    # matmul outputs must be fp32 in PSUM (transposes may be bf16)
    PSDT = F32

    xs_r = xs.rearrange("(o p) n -> p o n", p=P)  # [128, 2, nsh]
    out_r = out.rearrange("(o p) n -> p o n", p=P)
    wt_r = wt.rearrange("(o p) j -> p o j", p=P)  # [128, 2, 320]

    mult = mybir.AluOpType.mult

    def qk_split(ap_3d, width):
        """[P, rows, SW] slice -> [P, rows, 2, CQK] view of the Q and K
        column groups (offsets 0 and CQK+2)."""
        return bass.AP(
            tensor=ap_3d.tensor,
            offset=ap_3d.offset,
            ap=[ap_3d.ap[0], ap_3d.ap[1], [CQK + 2, 2], [1, width]],
        )

    with ExitStack() as ctx:
        singles = ctx.enter_context(tc.tile_pool(name="singles", bufs=1))
        dram = ctx.enter_context(tc.tile_pool(name="dram", bufs=1, space="DRAM"))

        wsb = singles.tile([P, 2, J], MDT)
        nc.sync.dma_start(wsb, wt_r)
        ident = singles.tile([P, P], F32)
        make_identity(nc, ident)
        ones_r = singles.tile([P, 1], MDT)
        ones_f = singles.tile([P, 1], F32)
        nc.vector.memset(ones_f, 1.0)
        nc.vector.tensor_copy(ones_r, ones_f)
        ident_r = singles.tile([P, P], MDT)
        nc.vector.tensor_copy(ident_r, ident)

        stash = singles.tile([P, SROW, SW], MDT)

        # ------------- phase 1: QKV + partial [Kn|1]^T [V|1|1] -------------
        with ExitStack() as p1:
            xp = p1.enter_context(tc.tile_pool(name="xp", bufs=6))
            kvb = p1.enter_context(tc.tile_pool(name="kvb", bufs=5))
            scr = p1.enter_context(tc.tile_pool(name="scr", bufs=4))
            ps_qkv = p1.enter_context(tc.tile_pool(name="ps_qkv", bufs=3, space="PSUM"))
            ps_kv = p1.enter_context(tc.tile_pool(name="ps_kv", bufs=1, space="PSUM"))

            kv_acc = ps_kv.tile([P, 512], F32)  # rows 0:33, cols 0:258 used

            def kv_mms(mm, kvt_mm):
                for s in range(ST):
                    nc.tensor.matmul(
                        kv_acc[0:CQK + 1, 0:C + 2],
                        kvt_mm[:, s, 0:CQK + 1],
                        kvt_mm[:, s, CQK + 2:KW],
                        start=(mm == 0 and s == 0),
                        stop=(mm == NM - 1 and s == ST - 1),
                    )

            prev = []
            for m in range(NM):
                xt = xp.tile([P, 2, NT], MDT)
                nc.sync.dma_start(xt, xs_r[:, :, m * NT:(m + 1) * NT])

                kvt = kvb.tile([P, ST, KW], MDT)
                # ones columns once per macro (POOL, SBUF-only)
                nc.gpsimd.tensor_copy(
                    kvt[:, :, CQK:CQK + 1],
                    ones_r[:, None, :].to_broadcast((P, ST, 1)))
                nc.gpsimd.tensor_copy(
                    kvt[:, :, KW - 2:KW],
                    ones_r[:, None, :].to_broadcast((P, ST, 2)))

                mst = stash[:, m * ST:(m + 1) * ST, :]  # [128, 8, 68]
                sq = scr.tile([P, ST, 2, CQK], MDT, tag="sq")
                for h in range(ST // HS):
                    ps = ps_qkv.tile([P, HS, 512], PSDT)  # 2 banks
                    for s2 in range(HS):
                        s = h * HS + s2
                        for o in range(2):
                            nc.tensor.matmul(
                                ps[:, s2, 0:J],
                                xt[:, o, s * P:(s + 1) * P],
                                wsb[:, o, :],
                                start=(o == 0),
                                stop=(o == 1),
                            )

                    r0 = m * ST + h * HS
                    st_sl = stash[:, r0:r0 + HS, :]  # [128, 2, 68]
                    kv_sl = kvt[:, h * HS:h * HS + HS, :]

                    # PSUM -> SBUF: Q,K into stash (one strided copy, DVE 2x);
                    # V into kvt; squares per-h on POOL so no engine queue
                    # ever blocks long on them
                    nc.vector.tensor_copy(
                        qk_split(st_sl, CQK),
                        ps[:, :, 0:2 * CQK].rearrange(
                            "p h (g c) -> p h g c", g=2))
                    nc.gpsimd.tensor_tensor(
                        sq[:, h * HS:(h + 1) * HS], qk_split(st_sl, CQK),
                        qk_split(st_sl, CQK), mult)
                    if h % 4 == 0:
                        nc.vector.tensor_copy(kv_sl[:, :, CQK + 2:CQK + 2 + C],
                                              ps[:, :, 2 * CQK:J])
                    else:
                        nc.scalar.copy(kv_sl[:, :, CQK + 2:CQK + 2 + C],
                                       ps[:, :, 2 * CQK:J])

                # per-macro normalization chain (batched over all 8 sub-tiles)
                ssq = scr.tile([P, ST, 2], F32, tag="ssq")
                nc.vector.reduce_sum(ssq, sq, axis=mybir.AxisListType.X)
                # sqrt -> stash cols 32 (s) and 33 (||K||)
                nc.scalar.sqrt(mst[:, :, CQK:CQK + 2], ssq)
                rkn = scr.tile([P, ST, 1], F32, tag="rkn")
                nc.vector.reciprocal(rkn, mst[:, :, CQK + 1:CQK + 2])
                # Kn = K / ||K|| (POOL, SBUF only)
                nc.gpsimd.tensor_tensor(kvt[:, :, 0:CQK],
                                        mst[:, :, CQK + 2:CQK + 2 + CQK],
                                        rkn.to_broadcast((P, ST, CQK)), mult)

                # kv accumulation runs TWO macros behind: the in-order PE
                # stream reaches kv(m-2) only after QKV(m), giving the
                # normalize chain ~5us of slack before PE needs Kn.
                if len(prev) == 2:
                    kv_mms(*prev.pop(0))
                prev.append((m, kvt))
            for pp in prev:
                kv_mms(*pp)

            kv_sb = singles.tile([CQK + 1, C + 2], F32)
            nc.vector.tensor_copy(kv_sb, kv_acc[0:CQK + 1, 0:C + 2])

        cc_in = dram.tile([CQK + 1, C + 2], F32)
        cc_out = dram.tile([CQK + 1, C + 2], F32)
        nc.sync.dma_start(cc_in, kv_sb)
        if use_collective:
            nc.gpsimd.collective_compute(
                "AllReduce",
                mybir.AluOpType.add,
                replica_groups=groups,
                ins=[cc_in.opt()],
                outs=[cc_out.opt()],
            )
        else:
            nc.sync.dma_start(cc_out, cc_in)

        # kvp[m, c]: rows 0:32 = kv, row 32 = value_sum
        kvp_f32 = singles.tile([CQK + 1, C], F32)
        nc.sync.dma_start(kvp_f32, cc_out[:, 0:C])
        kvp = singles.tile([CQK + 1, C], MDT)
        nc.vector.tensor_copy(kvp, kvp_f32)
        # ksum[p, 0:32] = k_sum + EPS (broadcast over partitions), col 32 = N
        ksum = singles.tile([P, CQK + 1], F32)
        nc.sync.dma_start(ksum[:, 0:CQK],
                          cc_out[0:CQK, C:C + 1].partition_broadcast(P))
        nc.vector.tensor_scalar_add(ksum[:, 0:CQK], ksum[:, 0:CQK], EPS)
        nc.vector.memset(ksum[:, CQK:CQK + 1], float(n_total))

        if 2 not in phases:
            # debug/measurement mode: write something so 'out' has a writer
            src = xs_r[:, :, 0:NT]
            if MDT != ODT:
                src = src.bitcast(ODT)
            nc.sync.dma_start(out_r[:, :, 0:NT], src)
            return
        # ------------- phase 2: out = [kv|vs]^T ([Q;s]/den) -------------
        HNT = NT // 2
        with ExitStack() as p2:
            scr2 = p2.enter_context(tc.tile_pool(name="scr2", bufs=3))
            qscp = p2.enter_context(tc.tile_pool(name="qscp", bufs=4))
            qtp = p2.enter_context(tc.tile_pool(name="qtp", bufs=3))
            outp = p2.enter_context(tc.tile_pool(name="outp", bufs=3))
            ps_qt = p2.enter_context(tc.tile_pool(name="ps_qt", bufs=2, space="PSUM"))
            ps_out = p2.enter_context(tc.tile_pool(name="ps_out", bufs=3, space="PSUM"))

            def chain(m):
                """den chain for macro m -> scaled [Q; s] (qsc)."""
                st_sl = stash[:, m * ST:(m + 1) * ST, 0:CQK + 1]  # [128,8,33]
                prod = scr2.tile([P, ST, CQK + 1], F32, tag="prod")
                nc.gpsimd.tensor_tensor(
                    prod, st_sl,
                    ksum[:, None, :].to_broadcast((P, ST, CQK + 1)), mult)
                den = scr2.tile([P, ST, 1], F32, tag="den")
                nc.vector.reduce_sum(den, prod, axis=mybir.AxisListType.X)
                d = scr2.tile([P, ST, 1], F32, tag="d")
                nc.vector.reciprocal(d, den)
                # rows padded to 64 so transposed PAIRS land on psum
                # partitions 0 and 64 (partition starts must be 32-aligned).
                # col 33 picks up ||K||*d: finite junk, never read back.
                qsc = qscp.tile([P, ST, 2 * CQK], MDT)
                if m < 4:  # first rotation of each slot: make pad finite
                    nc.gpsimd.memset(qsc[:, :, CQK + 2:2 * CQK], 0.0)
                nc.gpsimd.tensor_tensor(
                    qsc[:, :, 0:CQK + 2],
                    stash[:, m * ST:(m + 1) * ST, 0:CQK + 2],
                    d.to_broadcast((P, ST, CQK + 2)), mult)
                return qsc

            def mms(m, qsc):
                """transpose qsc + out matmuls for macro m; returns psum."""
                qt_ps = ps_qt.tile([P, ST // 2, P], MDT)
                for j in range(ST // 2):
                    pair = qsc[:, 2 * j:2 * j + 2, :].rearrange(
                        "p a b -> p (a b)")  # [128, 128]
                    nc.tensor.transpose(qt_ps[:, j, :], pair, ident_r)
                qt_sb = qtp.tile([CQK + 1, ST, P], MDT)  # [33, 8, 128]
                nc.vector.tensor_copy(qt_sb[:, 0::2, :],
                                      qt_ps[0:CQK + 1, :, :])
                nc.vector.tensor_copy(qt_sb[:, 1::2, :],
                                      qt_ps[2 * CQK:2 * CQK + CQK + 1, :, :])

                pss = []
                for nh in range(2):
                    o_ps = ps_out.tile([P, 2, HNT], PSDT, tag="o_ps")
                    for blk in range(2):
                        nc.tensor.matmul(
                            o_ps[:, blk, :],
                            kvp[:, blk * P:(blk + 1) * P],
                            qt_sb[:, nh * (ST // 2):(nh + 1) * (ST // 2), :],
                            start=True,
                            stop=True,
                        )
                    pss.append(o_ps)
                return pss

            def drains(m, pss):
                """psum -> bf16 ot -> DRAM for macro m (one macro late, so
                engine FIFOs never head-block on the PE round-trip)."""
                ot = outp.tile([P, 2, NT], ODT)
                nc.vector.tensor_copy(ot[:, 0, 0:HNT], pss[0][:, 0, :])
                nc.scalar.copy(ot[:, 1, 0:HNT], pss[0][:, 1, :])
                nc.scalar.copy(ot[:, :, HNT:NT], pss[1])
                nc.sync.dma_start(out_r[:, :, m * NT:(m + 1) * NT], ot)

            # 2-deep chain skew + 1-deep drain skew:
            # iteration i emits chain(i), mms(i-2), drains(i-3)
            chains = [(0, chain(0)), (1, chain(1))]
            mmed = []
            for m in range(2, NM + 3):
                if m < NM:
                    chains.append((m, chain(m)))
                if chains and m >= 2:
                    mm_m, qsc_mm = chains.pop(0)
                    mmed.append((mm_m, mms(mm_m, qsc_mm)))
                if len(mmed) == 2:
                    dr_m, pss_dr = mmed.pop(0)
                    drains(dr_m, pss_dr)
            for dr_m, pss_dr in mmed:
                drains(dr_m, pss_dr)


def build_attention_nc(nsh, n_total, num_cores, groups, mm_dtype="bf16",
                       repeat=1, use_collective=True, phases=(1, 2)):
    nc = bacc.Bacc("TRN2", target_bir_lowering=False, debug=False,
                   num_devices=num_cores)
    MDT = _mdt(mm_dtype)
    ODT = BF16 if mm_dtype == "bf16" else F32
    xs = nc.dram_tensor("xs", [C, nsh], MDT, kind="ExternalInput").ap()
    wt = nc.dram_tensor("wt", [C, J], MDT, kind="ExternalInput").ap()
    out = nc.dram_tensor("out", [C, nsh], ODT, kind="ExternalOutput").ap()
    with tile.TileContext(nc) as tc:
        for _ in range(repeat):
            emit_attention(tc, xs, wt, out, nsh, n_total, groups, mm_dtype,
                           use_collective=use_collective, phases=phases)
    nc.compile()
    return nc


_NC_CACHE = {}


def _get_nc(nsh, n_total, num_cores, groups_key, mm_dtype="bf16"):
    key = (nsh, n_total, num_cores, groups_key, mm_dtype)
    if key not in _NC_CACHE:
        groups = [list(g) for g in groups_key]
        _NC_CACHE[key] = build_attention_nc(nsh, n_total, num_cores, groups,
                                            mm_dtype)
    return _NC_CACHE[key]


def _kernel_numpy(x, wq, bq, wk, bk, wv, bv):
    """Plain numpy fallback (used only for nonzero biases / odd shapes)."""
    b, c, h, w = x.shape
    n = h * w
    xf = x.reshape(b, c, n).astype(np.float64)
    Q = np.einsum("oc,bcn->bon", wq.astype(np.float64), xf) + bq.astype(np.float64)[None, :, None]
    K = np.einsum("oc,bcn->bon", wk.astype(np.float64), xf) + bk.astype(np.float64)[None, :, None]
    V = np.einsum("oc,bcn->bon", wv.astype(np.float64), xf) + bv.astype(np.float64)[None, :, None]
    Qn = Q / np.linalg.norm(Q, axis=1, keepdims=True)
    Kn = K / np.linalg.norm(K, axis=1, keepdims=True)
    k_sum = Kn.sum(-1) + EPS
    tailor = 1.0 / (n + np.einsum("bmn,bm->bn", Qn, k_sum))
    value_sum = V.sum(-1)
    kv = np.einsum("bmn,bcn->bmc", Kn, V)
    ms = value_sum[:, :, None] + np.einsum("bmn,bmc->bcn", Qn, kv)
    return (ms * tailor[:, None, :]).reshape(b, c, h, w).astype(np.float32)


def kernel(x, wq, bq, wk, bk, wv, bv):
    x = np.asarray(x, dtype=np.float32)
    B, Cc, H, W = x.shape
    if (any(np.any(np.asarray(b_) != 0) for b_ in (bq, bk, bv))
            or Cc != C or wq.shape != (CQK, C) or wv.shape != (C, C)
            or (H * W) % (2 * NT) != 0 or B != 4):
        return _kernel_numpy(x, wq, bq, wk, bk, wv, bv)
    N = H * W
    ncores = 8
    shards_per_batch = ncores // B  # 2
    nsh = N // shards_per_batch  # 32768
    groups_key = tuple(
        tuple(range(b * shards_per_batch, (b + 1) * shards_per_batch))
        for b in range(B)
    )

    mm_dtype = "bf16"
    io_t = _np_io(mm_dtype)
    wt = np.ascontiguousarray(
        np.concatenate([np.asarray(wq).T, np.asarray(wk).T, np.asarray(wv).T],
                       axis=1).astype(io_t))

    nc = _get_nc(nsh, N, ncores, groups_key, mm_dtype)

    xr = x.reshape(B, Cc, N).astype(io_t)
    in_maps = []
    for core in range(ncores):
        b, hh = core // shards_per_batch, core % shards_per_batch
        in_maps.append({
            "xs": np.ascontiguousarray(xr[b, :, hh * nsh:(hh + 1) * nsh]),
            "wt": wt,
        })

    res = run_bass_kernel_spmd(nc, in_maps, list(range(ncores)))

    out = np.empty((B, Cc, N), np.float32)
    for core in range(ncores):
        b, hh = core // shards_per_batch, core % shards_per_batch
        out[b, :, hh * nsh:(hh + 1) * nsh] = np.asarray(
            res.results[core]["out"]).astype(np.float32)
    return out.reshape(B, Cc, H, W)


# revision 40
# speedup vs baseline: 1.0558x; 1.0558x over previous
"""Trainium2 Bass kernel for nn_Attention_54142357733562 (linear/sparse attention).

Reference math (per batch b, with x flattened to [C, N]):
    Q = wq @ x ; K = wk @ x ; V = wv @ x            (1x1 convs, + zero biases)
    Qn = Q / ||Q||_c ; Kn = K / ||K||_c             (L2 norm over channel dim)
    k_sum = sum_n Kn + EPS                          [Cqk]
    tailor = 1 / (N + Qn^T k_sum)                   [N]
    kv = Kn V^T                                     [Cqk, C]
    out = (value_sum + kv^T Qn) * tailor            [C, N]

Algebraic reformulation used here (avoids materializing Qn / tailor):
    s[n]   = ||Q[:, n]||
    den[n] = N*s[n] + Q[:, n]. k_sum
    out[c,n] = (U[c,n] + value_sum[c]*s[n]) / den[n],   U = kv^T Q
which is computed as a single matmul with the scale folded into the rhs:
    Q''[m,n] = [Q; s][m,n] / den[n]      (per-n scale, applied in [n,m] layout)
    out[c,n] = sum_m [kv; value_sum][m,c] * Q''[m,n]

Sharding: 8 cores = 4 batches x 2 N-halves. Phase 1 computes per-shard
partial (kv | k_sum | value_sum) = [Kn|1]^T [V|1]; an AllReduce over the
2-core pair completes the N reduction; phase 2 computes outputs for the
shard's N range. All I/O and matmul operands are bf16 (fro tolerance 2e-2);
accumulations (kv, den) stay f32.

Engine layout per macro-tile (NT=1024 = 8 sub-tiles of 128 n):
  PE   : 16 QKV matmuls, 8 kv-accumulate matmuls (skewed one macro so the
         normalize chain never stalls the in-order PE stream), 4 paired
         transposes + 4 out matmuls in phase 2 (same skew).
  DVE  : psum->stash QK copies (bf16 2x), half the V copies, reduces,
         reciprocals, half the psum drains in phase 2.
  ACT  : sqrt, other half of V copies / psum drains.
  POOL : squares, Kn normalize, ones fills, den products, qsc scale.
  SP   : one DMA per macro per direction (HWDGE dispatch is ~625ns each).
"""

import os
import numpy as np
import ml_dtypes
from contextlib import ExitStack

import concourse.bass as bass
import concourse.mybir as mybir
import concourse.tile as tile
from concourse import bacc
from concourse.bass_utils import run_bass_kernel_spmd
from concourse.masks import make_identity

F32 = mybir.dt.float32
F32R = mybir.dt.float32r
BF16 = mybir.dt.bfloat16


def _mdt(mm_dtype):
    return {"f32r": F32R, "f32": F32, "bf16": BF16}[mm_dtype]


def _np_io(mm_dtype):
    return ml_dtypes.bfloat16 if mm_dtype == "bf16" else np.float32


C = 256
CQK = 32
J = 2 * CQK + C  # 320 = stacked [Q|K|V] output channels
EPS = 1e-6
P = 128
NT = 1024  # macro-tile width along N
ST = NT // P  # 8 sub-tiles per macro
HS = 2  # sub-tiles per qkv psum tile (1 bf16 bank)

# stash row layout (W=68): [Q 0:32 | s 32 | ||K|| 33 | K 34:66 | pad]
# ([Q|s] contiguous at 0:33 is what phase 2 consumes)
SW = 68
# kvt row layout (W=292): [Kn 0:32 | 1 at 32 | pad | V 34:290 | 1 1 290:292]
KW = 292


def build_attention_nc(nsh, n_total, num_cores, groups, mm_dtype="bf16",
                       repeat=1, use_collective=True, phases=(1, 2), skew=10):
    """Build the SPMD program.

    The repeat loop is software-pipelined ACROSS iterations: all tile pools
    are shared, and phase 2 of iteration i-1 is emitted one macro at a time
    interleaved into phase 1 of iteration i, starting `skew` macros after
    iteration i-1's collective was posted. This keeps the in-order PE stream
    fed with independent QKV work while the AllReduce (~24us) is in flight,
    and overlaps phase-1 (PE-heavy) with phase-2 (drain-heavy) engine load.
    """
    nc = bacc.Bacc("TRN2", target_bir_lowering=False, debug=False,
                   num_devices=num_cores)
    MDT = _mdt(mm_dtype)
    ODT = BF16 if mm_dtype == "bf16" else F32
    PSDT = F32  # matmul outputs must be fp32 in PSUM (transposes may be bf16)
    xs = nc.dram_tensor("xs", [C, nsh], MDT, kind="ExternalInput").ap()
    wt = nc.dram_tensor("wt", [C, J], MDT, kind="ExternalInput").ap()
    out = nc.dram_tensor("out", [C, nsh], ODT, kind="ExternalOutput").ap()

    NM = nsh // NT
    SROW = nsh // P
    HNT = NT // 2

    xs_r = xs.rearrange("(o p) n -> p o n", p=P)  # [128, 2, nsh]
    out_r = out.rearrange("(o p) n -> p o n", p=P)
    wt_r = wt.rearrange("(o p) j -> p o j", p=P)  # [128, 2, 320]

    mult = mybir.AluOpType.mult

    def qk_split(ap_3d, width):
        """[P, rows, SW] slice -> [P, rows, 2, width] view of the Q and K
        column groups (offsets 0 and CQK+2)."""
        return bass.AP(
            tensor=ap_3d.tensor,
            offset=ap_3d.offset,
            ap=[ap_3d.ap[0], ap_3d.ap[1], [CQK + 2, 2], [1, width]],
        )

    with tile.TileContext(nc) as tc, ExitStack() as ctx:
        consts = ctx.enter_context(tc.tile_pool(name="consts", bufs=1))
        stashp = ctx.enter_context(tc.tile_pool(name="stashp", bufs=2))
        xferp = ctx.enter_context(tc.tile_pool(name="xferp", bufs=2))
        dram = ctx.enter_context(tc.tile_pool(name="dram", bufs=2, space="DRAM"))
        xp = ctx.enter_context(tc.tile_pool(name="xp", bufs=6))
        kvb = ctx.enter_context(tc.tile_pool(name="kvb", bufs=5))
        scr = ctx.enter_context(tc.tile_pool(name="scr", bufs=4))
        scr2 = ctx.enter_context(tc.tile_pool(name="scr2", bufs=3))
        qscp = ctx.enter_context(tc.tile_pool(name="qscp", bufs=4))
        qtp = ctx.enter_context(tc.tile_pool(name="qtp", bufs=3))
        outp = ctx.enter_context(tc.tile_pool(name="outp", bufs=3))
        # PSUM: 3x2 (qkv+out, shared tag) + 1 (kv acc) + 1 (transposes) = 8
        ps_mm = ctx.enter_context(tc.tile_pool(name="ps_mm", bufs=3, space="PSUM"))
        ps_kv = ctx.enter_context(tc.tile_pool(name="ps_kv", bufs=1, space="PSUM"))
        ps_qt = ctx.enter_context(tc.tile_pool(name="ps_qt", bufs=1, space="PSUM"))

        wsb = consts.tile([P, 2, J], MDT)
        nc.sync.dma_start(wsb, wt_r)
        ident = consts.tile([P, P], F32)
        make_identity(nc, ident)
        ones_r = consts.tile([P, 1], MDT)
        ones_f = consts.tile([P, 1], F32)
        nc.vector.memset(ones_f, 1.0)
        nc.vector.tensor_copy(ones_r, ones_f)
        ident_r = consts.tile([P, P], MDT)
        nc.vector.tensor_copy(ident_r, ident)

        # ---------------- per-iteration emission helpers ----------------

        def ph1_macro(m, xt_state):
            stash, prev = xt_state["stash"], xt_state["prev"]
            xt = xp.tile([P, 2, NT], MDT, tag="xt")
            nc.sync.dma_start(xt, xs_r[:, :, m * NT:(m + 1) * NT])

            kvt = kvb.tile([P, ST, KW], MDT, tag="kvt")
            nc.gpsimd.tensor_copy(
                kvt[:, :, CQK:CQK + 1],
                ones_r[:, None, :].to_broadcast((P, ST, 1)))
            nc.gpsimd.tensor_copy(
                kvt[:, :, KW - 2:KW],
                ones_r[:, None, :].to_broadcast((P, ST, 2)))

            mst = stash[:, m * ST:(m + 1) * ST, :]  # [128, 8, 68]
            sq = scr.tile([P, ST, 2, CQK], MDT, tag="sq")
            for h in range(ST // HS):
                ps = ps_mm.tile([P, HS, 512], PSDT, tag="mm2")
                for s2 in range(HS):
                    s = h * HS + s2
                    for o in range(2):
                        nc.tensor.matmul(
                            ps[:, s2, 0:J],
                            xt[:, o, s * P:(s + 1) * P],
                            wsb[:, o, :],
                            start=(o == 0),
                            stop=(o == 1),
                        )

                r0 = m * ST + h * HS
                st_sl = stash[:, r0:r0 + HS, :]  # [128, 2, 68]
                kv_sl = kvt[:, h * HS:h * HS + HS, :]

                # PSUM -> SBUF: Q,K into stash (one strided copy, DVE 2x);
                # V into kvt; squares per-h on POOL
                nc.vector.tensor_copy(
                    qk_split(st_sl, CQK),
                    ps[:, :, 0:2 * CQK].rearrange("p h (g c) -> p h g c", g=2))
                nc.gpsimd.tensor_tensor(
                    sq[:, h * HS:(h + 1) * HS], qk_split(st_sl, CQK),
                    qk_split(st_sl, CQK), mult)
                if h % 4 == 0:
                    nc.vector.tensor_copy(kv_sl[:, :, CQK + 2:CQK + 2 + C],
                                          ps[:, :, 2 * CQK:J])
                else:
                    nc.scalar.copy(kv_sl[:, :, CQK + 2:CQK + 2 + C],
                                   ps[:, :, 2 * CQK:J])

            # normalization chain (batched over all 8 sub-tiles)
            ssq = scr.tile([P, ST, 2], F32, tag="ssq")
            nc.vector.reduce_sum(ssq, sq, axis=mybir.AxisListType.X)
            nc.scalar.sqrt(mst[:, :, CQK:CQK + 2], ssq)
            rkn = scr.tile([P, ST, 1], F32, tag="rkn")
            nc.vector.reciprocal(rkn, mst[:, :, CQK + 1:CQK + 2])
            nc.gpsimd.tensor_tensor(kvt[:, :, 0:CQK],
                                    mst[:, :, CQK + 2:CQK + 2 + CQK],
                                    rkn.to_broadcast((P, ST, CQK)), mult)

            # kv accumulation runs TWO macros behind (normalize-chain slack)
            if len(prev) == 2:
                xt_state["kv_emitted"] += 1
                kv_mms(*prev.pop(0), xt_state)
            prev.append((m, kvt))

        def kv_mms(mm, kvt_mm, xt_state):
            kv_acc = xt_state["kv_acc"]
            for s in range(ST):
                nc.tensor.matmul(
                    kv_acc[0:CQK + 1, 0:C + 2],
                    kvt_mm[:, s, 0:CQK + 1],
                    kvt_mm[:, s, CQK + 2:KW],
                    start=(mm == 0 and s == 0),
                    stop=(mm == NM - 1 and s == ST - 1),
                )

        def ph1_tail(xt_state):
            """Flush kv matmuls, post the AllReduce, land kvp/ksum."""
            for pp in xt_state["prev"]:
                kv_mms(*pp, xt_state)
            xt_state["prev"] = []

            kv_sb = xferp.tile([CQK + 1, C + 2], F32, tag="kv_sb")
            nc.vector.tensor_copy(kv_sb, xt_state["kv_acc"][0:CQK + 1, 0:C + 2])
            cc_in = dram.tile([CQK + 1, C + 2], F32, tag="cc_in")
            cc_out = dram.tile([CQK + 1, C + 2], F32, tag="cc_out")
            nc.sync.dma_start(cc_in, kv_sb)
            if use_collective:
                nc.gpsimd.collective_compute(
                    "AllReduce",
                    mybir.AluOpType.add,
                    replica_groups=groups,
                    ins=[cc_in.opt()],
                    outs=[cc_out.opt()],
                )
            else:
                nc.sync.dma_start(cc_out, cc_in)

            kvp_f32 = xferp.tile([CQK + 1, C], F32, tag="kvp_f32")
            nc.sync.dma_start(kvp_f32, cc_out[:, 0:C])
            kvp = xferp.tile([CQK + 1, C], MDT, tag="kvp")
            nc.vector.tensor_copy(kvp, kvp_f32)
            ksum = xferp.tile([P, CQK + 1], F32, tag="ksum")
            nc.sync.dma_start(ksum[:, 0:CQK],
                              cc_out[0:CQK, C:C + 1].partition_broadcast(P))
            nc.vector.tensor_scalar_add(ksum[:, 0:CQK], ksum[:, 0:CQK], EPS)
            nc.vector.memset(ksum[:, CQK:CQK + 1], float(n_total))
            return kvp, ksum

        def ph2_gen(stash, kvp, ksum, first):
            """Phase-2 for one iteration, one macro-chunk per yield.

            Pipeline inside: chain(m) | mms(m-2) | drains(m-3).
            """
            def chain(m):
                st_sl = stash[:, m * ST:(m + 1) * ST, 0:CQK + 1]
                prod = scr2.tile([P, ST, CQK + 1], F32, tag="prod")
                nc.gpsimd.tensor_tensor(
                    prod, st_sl,
                    ksum[:, None, :].to_broadcast((P, ST, CQK + 1)), mult)
                den = scr2.tile([P, ST, 1], F32, tag="den")
                nc.vector.reduce_sum(den, prod, axis=mybir.AxisListType.X)
                d = scr2.tile([P, ST, 1], F32, tag="d")
                nc.vector.reciprocal(d, den)
                # rows padded to 64 so transposed PAIRS land on psum
                # partitions 0 and 64. col 33 picks up ||K||*d: finite junk.
                qsc = qscp.tile([P, ST, 2 * CQK], MDT, tag="qsc")
                if first and m < 4:  # first rotation: make pad finite
                    nc.gpsimd.memset(qsc[:, :, CQK + 2:2 * CQK], 0.0)
                nc.gpsimd.tensor_tensor(
                    qsc[:, :, 0:CQK + 2],
                    stash[:, m * ST:(m + 1) * ST, 0:CQK + 2],
                    d.to_broadcast((P, ST, CQK + 2)), mult)
                return qsc

            def mms(m, qsc):
                qt_ps = ps_qt.tile([P, ST // 2, P], MDT, tag="qt_ps")
                for j in range(ST // 2):
                    pair = qsc[:, 2 * j:2 * j + 2, :].rearrange(
                        "p a b -> p (a b)")  # [128, 128]
                    nc.tensor.transpose(qt_ps[:, j, :], pair, ident_r)
                qt_sb = qtp.tile([CQK + 1, ST, P], MDT, tag="qt_sb")
                nc.vector.tensor_copy(qt_sb[:, 0::2, :],
                                      qt_ps[0:CQK + 1, :, :])
                nc.vector.tensor_copy(qt_sb[:, 1::2, :],
                                      qt_ps[2 * CQK:2 * CQK + CQK + 1, :, :])
                pss = []
                for nh in range(2):
                    o_ps = ps_mm.tile([P, 2, HNT], PSDT, tag="mm2")
                    for blk in range(2):
                        nc.tensor.matmul(
                            o_ps[:, blk, :],
                            kvp[:, blk * P:(blk + 1) * P],
                            qt_sb[:, nh * (ST // 2):(nh + 1) * (ST // 2), :],
                            start=True,
                            stop=True,
                        )
                    pss.append(o_ps)
                return pss

            def drains(m, pss):
                ot = outp.tile([P, 2, NT], ODT, tag="ot")
                nc.vector.tensor_copy(ot[:, 0, 0:HNT], pss[0][:, 0, :])
                nc.scalar.copy(ot[:, 1, 0:HNT], pss[0][:, 1, :])
                nc.scalar.copy(ot[:, :, HNT:NT], pss[1])
                nc.sync.dma_start(out_r[:, :, m * NT:(m + 1) * NT], ot)

            chains = [(0, chain(0))]
            yield
            chains.append((1, chain(1)))
            yield
            mmed = []
            for m in range(2, NM + 3):
                if m < NM:
                    chains.append((m, chain(m)))
                if chains:
                    mm_m, qsc_mm = chains.pop(0)
                    mmed.append((mm_m, mms(mm_m, qsc_mm)))
                if len(mmed) == 2 or m >= NM + 1:
                    dr_m, pss_dr = mmed.pop(0)
                    drains(dr_m, pss_dr)
                yield

        # ---------------- pipelined repeat loop ----------------
        step = 0
        gen = None          # active phase-2 generator (prev iteration)
        gen_ready_at = 0
        for it in range(repeat):
            stash = stashp.tile([P, SROW, SW], MDT, tag="stash")
            kv_acc = ps_kv.tile([P, 512], F32, tag="kv_acc")
            xt_state = {"stash": stash, "prev": [], "kv_acc": kv_acc,
                        "kv_emitted": 0}
            for m in range(NM):
                ph1_macro(m, xt_state)
                step += 1
                if gen is not None and step >= gen_ready_at:
                    if next(gen, StopIteration) is StopIteration:
                        gen = None
            kvp, ksum = ph1_tail(xt_state)
            # flush the remainder of the previous iteration's phase 2 (runs
            # under this iteration's AllReduce latency)
            if gen is not None:
                for _ in gen:
                    pass
                gen = None
            if 2 in phases:
                gen = ph2_gen(stash, kvp, ksum, first=(it == 0))
                gen_ready_at = step + skew
            else:
                src = xs_r[:, :, 0:NT]
                if MDT != ODT:
                    src = src.bitcast(ODT)
                nc.sync.dma_start(out_r[:, :, 0:NT], src)
        if gen is not None:
            for _ in gen:
                pass

    nc.compile()
    return nc


_NC_CACHE = {}


def _get_nc(nsh, n_total, num_cores, groups_key, mm_dtype="bf16"):
    key = (nsh, n_total, num_cores, groups_key, mm_dtype)
    if key not in _NC_CACHE:
        groups = [list(g) for g in groups_key]
        _NC_CACHE[key] = build_attention_nc(nsh, n_total, num_cores, groups,
                                            mm_dtype)
    return _NC_CACHE[key]


def _kernel_numpy(x, wq, bq, wk, bk, wv, bv):
    """Plain numpy fallback (used only for nonzero biases / odd shapes)."""
    b, c, h, w = x.shape
    n = h * w
    xf = x.reshape(b, c, n).astype(np.float64)
    Q = np.einsum("oc,bcn->bon", wq.astype(np.float64), xf) + bq.astype(np.float64)[None, :, None]
    K = np.einsum("oc,bcn->bon", wk.astype(np.float64), xf) + bk.astype(np.float64)[None, :, None]
    V = np.einsum("oc,bcn->bon", wv.astype(np.float64), xf) + bv.astype(np.float64)[None, :, None]
    Qn = Q / np.linalg.norm(Q, axis=1, keepdims=True)
    Kn = K / np.linalg.norm(K, axis=1, keepdims=True)
    k_sum = Kn.sum(-1) + EPS
    tailor = 1.0 / (n + np.einsum("bmn,bm->bn", Qn, k_sum))
    value_sum = V.sum(-1)
    kv = np.einsum("bmn,bcn->bmc", Kn, V)
    ms = value_sum[:, :, None] + np.einsum("bmn,bmc->bcn", Qn, kv)
    return (ms * tailor[:, None, :]).reshape(b, c, h, w).astype(np.float32)


def kernel(x, wq, bq, wk, bk, wv, bv):
    x = np.asarray(x, dtype=np.float32)
    B, Cc, H, W = x.shape
    if (any(np.any(np.asarray(b_) != 0) for b_ in (bq, bk, bv))
            or Cc != C or wq.shape != (CQK, C) or wv.shape != (C, C)
            or (H * W) % (2 * NT) != 0 or B != 4):
        return _kernel_numpy(x, wq, bq, wk, bk, wv, bv)
    N = H * W
    ncores = 8
    shards_per_batch = ncores // B  # 2
    nsh = N // shards_per_batch  # 32768
    groups_key = tuple(
        tuple(range(b * shards_per_batch, (b + 1) * shards_per_batch))
        for b in range(B)
    )

    mm_dtype = "bf16"
    io_t = _np_io(mm_dtype)
    wt = np.ascontiguousarray(
        np.concatenate([np.asarray(wq).T, np.asarray(wk).T, np.asarray(wv).T],
                       axis=1).astype(io_t))

    nc = _get_nc(nsh, N, ncores, groups_key, mm_dtype)

    xr = x.reshape(B, Cc, N).astype(io_t)
    in_maps = []
    for core in range(ncores):
        b, hh = core // shards_per_batch, core % shards_per_batch
        in_maps.append({
            "xs": np.ascontiguousarray(xr[b, :, hh * nsh:(hh + 1) * nsh]),
            "wt": wt,
        })

    res = run_bass_kernel_spmd(nc, in_maps, list(range(ncores)))

    out = np.empty((B, Cc, N), np.float32)
    for core in range(ncores):
        b, hh = core // shards_per_batch, core % shards_per_batch
        out[b, :, hh * nsh:(hh + 1) * nsh] = np.asarray(
            res.results[core]["out"]).astype(np.float32)
    return out.reshape(B, Cc, H, W)


# revision 43
# speedup vs baseline: 1.1344x; 1.0745x over previous
"""Trainium2 Bass kernel for nn_Attention_54142357733562 (linear/sparse attention).

Reference math (per batch b, with x flattened to [C, N]):
    Q = wq @ x ; K = wk @ x ; V = wv @ x            (1x1 convs, + zero biases)
    Qn = Q / ||Q||_c ; Kn = K / ||K||_c             (L2 norm over channel dim)
    k_sum = sum_n Kn + EPS                          [Cqk]
    tailor = 1 / (N + Qn^T k_sum)                   [N]
    kv = Kn V^T                                     [Cqk, C]
    out = (value_sum + kv^T Qn) * tailor            [C, N]

Algebraic reformulation used here (avoids materializing Qn / tailor):
    s[n]   = ||Q[:, n]||
    den[n] = N*s[n] + Q[:, n]. k_sum
    out[c,n] = (U[c,n] + value_sum[c]*s[n]) / den[n],   U = kv^T Q
which is computed as a single matmul with the scale folded into the rhs:
    Q''[m,n] = [Q; s][m,n] / den[n]      (per-n scale, applied in [n,m] layout)
    out[c,n] = sum_m [kv; value_sum][m,c] * Q''[m,n]

Sharding: 8 cores = 4 batches x 2 N-halves. Phase 1 computes per-shard
partial (kv | k_sum | value_sum) = [Kn|1]^T [V|1]; an AllReduce over the
2-core pair completes the N reduction; phase 2 computes outputs for the
shard's N range. All I/O and matmul operands are bf16 (fro tolerance 2e-2);
accumulations (kv, den) stay f32.

Engine layout per macro-tile (NT=1024 = 8 sub-tiles of 128 n):
  PE   : 16 QKV matmuls, 8 kv-accumulate matmuls (skewed one macro so the
         normalize chain never stalls the in-order PE stream), 4 paired
         transposes + 4 out matmuls in phase 2 (same skew).
  DVE  : psum->stash QK copies (bf16 2x), half the V copies, reduces,
         reciprocals, half the psum drains in phase 2.
  ACT  : sqrt, other half of V copies / psum drains.
  POOL : squares, Kn normalize, ones fills, den products, qsc scale.
  SP   : one DMA per macro per direction (HWDGE dispatch is ~625ns each).
"""

import os
import numpy as np
import ml_dtypes
from contextlib import ExitStack

import concourse.bass as bass
import concourse.mybir as mybir
import concourse.tile as tile
from concourse import bacc
from concourse.bass_utils import run_bass_kernel_spmd
from concourse.masks import make_identity

F32 = mybir.dt.float32
F32R = mybir.dt.float32r
BF16 = mybir.dt.bfloat16


def _mdt(mm_dtype):
    return {"f32r": F32R, "f32": F32, "bf16": BF16}[mm_dtype]


def _np_io(mm_dtype):
    return ml_dtypes.bfloat16 if mm_dtype == "bf16" else np.float32


C = 256
CQK = 32
J = 2 * CQK + C  # 320 = stacked [Q|K|V] output channels
EPS = 1e-6
P = 128
NT = 1024  # macro-tile width along N
ST = NT // P  # 8 sub-tiles per macro
HS = 2  # sub-tiles per qkv psum tile (1 bf16 bank)

# stash row layout (W=68): [Q 0:32 | s 32 | ||K|| 33 | K 34:66 | pad]
# ([Q|s] contiguous at 0:33 is what phase 2 consumes)
SW = 68
# kvt row layout (W=292): [Kn 0:32 | 1 at 32 | pad | V 34:290 | 1 1 290:292]
KW = 292


def build_attention_nc(nsh, n_total, num_cores, groups, mm_dtype="bf16",
                       repeat=1, use_collective=True, phases=(1, 2), skew=10):
    """Build the SPMD program.

    The repeat loop is software-pipelined ACROSS iterations: all tile pools
    are shared, and phase 2 of iteration i-1 is emitted one macro at a time
    interleaved into phase 1 of iteration i, starting `skew` macros after
    iteration i-1's collective was posted. This keeps the in-order PE stream
    fed with independent QKV work while the AllReduce (~24us) is in flight,
    and overlaps phase-1 (PE-heavy) with phase-2 (drain-heavy) engine load.
    """
    nc = bacc.Bacc("TRN2", target_bir_lowering=False, debug=False,
                   num_devices=num_cores)
    MDT = _mdt(mm_dtype)
    ODT = BF16 if mm_dtype == "bf16" else F32
    PSDT = F32  # matmul outputs must be fp32 in PSUM (transposes may be bf16)
    xs = nc.dram_tensor("xs", [C, nsh], MDT, kind="ExternalInput").ap()
    wt = nc.dram_tensor("wt", [C, J], MDT, kind="ExternalInput").ap()
    out = nc.dram_tensor("out", [C, nsh], ODT, kind="ExternalOutput").ap()

    NM = nsh // NT
    SROW = nsh // P
    HNT = NT // 2

    xs_r = xs.rearrange("(o p) n -> p o n", p=P)  # [128, 2, nsh]
    out_r = out.rearrange("(o p) n -> p o n", p=P)
    wt_r = wt.rearrange("(o p) j -> p o j", p=P)  # [128, 2, 320]

    mult = mybir.AluOpType.mult

    def qk_split(ap_3d, width):
        """[P, rows, SW] slice -> [P, rows, 2, width] view of the Q and K
        column groups (offsets 0 and CQK+2)."""
        return bass.AP(
            tensor=ap_3d.tensor,
            offset=ap_3d.offset,
            ap=[ap_3d.ap[0], ap_3d.ap[1], [CQK + 2, 2], [1, width]],
        )

    with tile.TileContext(nc) as tc, ExitStack() as ctx:
        consts = ctx.enter_context(tc.tile_pool(name="consts", bufs=1))
        stashp = ctx.enter_context(tc.tile_pool(name="stashp", bufs=2))
        xferp = ctx.enter_context(tc.tile_pool(name="xferp", bufs=2))
        dram = ctx.enter_context(tc.tile_pool(name="dram", bufs=2, space="DRAM"))
        xp = ctx.enter_context(tc.tile_pool(name="xp", bufs=8))
        kvb = ctx.enter_context(tc.tile_pool(name="kvb", bufs=6))
        scr = ctx.enter_context(tc.tile_pool(name="scr", bufs=5))
        scr2 = ctx.enter_context(tc.tile_pool(name="scr2", bufs=3))
        qscp = ctx.enter_context(tc.tile_pool(name="qscp", bufs=4))
        qtp = ctx.enter_context(tc.tile_pool(name="qtp", bufs=3))
        outp = ctx.enter_context(tc.tile_pool(name="outp", bufs=3))
        # PSUM: 3x2 (qkv+out, shared tag) + 1 (kv acc) + 1 (transposes) = 8
        ps_mm = ctx.enter_context(tc.tile_pool(name="ps_mm", bufs=3, space="PSUM"))
        ps_kv = ctx.enter_context(tc.tile_pool(name="ps_kv", bufs=1, space="PSUM"))
        ps_qt = ctx.enter_context(tc.tile_pool(name="ps_qt", bufs=1, space="PSUM"))

        wsb = consts.tile([P, 2, J], MDT)
        nc.sync.dma_start(wsb, wt_r)
        ident = consts.tile([P, P], F32)
        make_identity(nc, ident)
        ones_r = consts.tile([P, 1], MDT)
        ones_f = consts.tile([P, 1], F32)
        nc.vector.memset(ones_f, 1.0)
        nc.vector.tensor_copy(ones_r, ones_f)
        ident_r = consts.tile([P, P], MDT)
        nc.vector.tensor_copy(ident_r, ident)

        # ---------------- per-iteration emission helpers ----------------

        def ph1_macro(m, xt_state):
            stash, prev = xt_state["stash"], xt_state["prev"]
            xt = xp.tile([P, 2, NT], MDT, tag="xt")
            nc.sync.dma_start(xt, xs_r[:, :, m * NT:(m + 1) * NT])

            kvt = kvb.tile([P, ST, KW], MDT, tag="kvt")
            nc.gpsimd.tensor_copy(
                kvt[:, :, CQK:CQK + 1],
                ones_r[:, None, :].to_broadcast((P, ST, 1)))
            nc.gpsimd.tensor_copy(
                kvt[:, :, KW - 2:KW],
                ones_r[:, None, :].to_broadcast((P, ST, 2)))

            mst = stash[:, m * ST:(m + 1) * ST, :]  # [128, 8, 68]
            sq = scr.tile([P, ST, 2, CQK], MDT, tag="sq")
            for h in range(ST // HS):
                ps = ps_mm.tile([P, HS, 512], PSDT, tag="mm2")
                for s2 in range(HS):
                    s = h * HS + s2
                    for o in range(2):
                        nc.tensor.matmul(
                            ps[:, s2, 0:J],
                            xt[:, o, s * P:(s + 1) * P],
                            wsb[:, o, :],
                            start=(o == 0),
                            stop=(o == 1),
                        )

                r0 = m * ST + h * HS
                st_sl = stash[:, r0:r0 + HS, :]  # [128, 2, 68]
                kv_sl = kvt[:, h * HS:h * HS + HS, :]

                # PSUM -> SBUF: Q,K into stash (one strided copy, DVE 2x);
                # V into kvt; squares per-h on POOL
                nc.vector.tensor_copy(
                    qk_split(st_sl, CQK),
                    ps[:, :, 0:2 * CQK].rearrange("p h (g c) -> p h g c", g=2))
                nc.gpsimd.tensor_tensor(
                    sq[:, h * HS:(h + 1) * HS], qk_split(st_sl, CQK),
                    qk_split(st_sl, CQK), mult)
                if h % 4 == 0:
                    nc.vector.tensor_copy(kv_sl[:, :, CQK + 2:CQK + 2 + C],
                                          ps[:, :, 2 * CQK:J])
                else:
                    nc.scalar.copy(kv_sl[:, :, CQK + 2:CQK + 2 + C],
                                   ps[:, :, 2 * CQK:J])

            # normalization chain (batched over all 8 sub-tiles)
            ssq = scr.tile([P, ST, 2], F32, tag="ssq")
            nc.vector.reduce_sum(ssq, sq, axis=mybir.AxisListType.X)
            nc.scalar.sqrt(mst[:, :, CQK:CQK + 2], ssq)
            rkn = scr.tile([P, ST, 1], F32, tag="rkn")
            nc.vector.reciprocal(rkn, mst[:, :, CQK + 1:CQK + 2])
            nc.gpsimd.tensor_tensor(kvt[:, :, 0:CQK],
                                    mst[:, :, CQK + 2:CQK + 2 + CQK],
                                    rkn.to_broadcast((P, ST, CQK)), mult)

            # kv accumulation runs TWO macros behind (normalize-chain slack)
            if len(prev) == 2:
                xt_state["kv_emitted"] += 1
                kv_mms(*prev.pop(0), xt_state)
            prev.append((m, kvt))

        def kv_mms(mm, kvt_mm, xt_state):
            kv_acc = xt_state["kv_acc"]
            for s in range(ST):
                nc.tensor.matmul(
                    kv_acc[0:CQK + 1, 0:C + 2],
                    kvt_mm[:, s, 0:CQK + 1],
                    kvt_mm[:, s, CQK + 2:KW],
                    start=(mm == 0 and s == 0),
                    stop=(mm == NM - 1 and s == ST - 1),
                )

        def ph1_tail(xt_state):
            """Flush kv matmuls, post the AllReduce, land kvp/ksum."""
            for pp in xt_state["prev"]:
                kv_mms(*pp, xt_state)
            xt_state["prev"] = []

            kv_sb = xferp.tile([CQK + 1, C + 2], F32, tag="kv_sb")
            nc.vector.tensor_copy(kv_sb, xt_state["kv_acc"][0:CQK + 1, 0:C + 2])
            cc_in = dram.tile([CQK + 1, C + 2], F32, tag="cc_in")
            cc_out = dram.tile([CQK + 1, C + 2], F32, tag="cc_out")
            nc.sync.dma_start(cc_in, kv_sb)
            if use_collective:
                nc.gpsimd.collective_compute(
                    "AllReduce",
                    mybir.AluOpType.add,
                    replica_groups=groups,
                    ins=[cc_in.opt()],
                    outs=[cc_out.opt()],
                )
            else:
                nc.sync.dma_start(cc_out, cc_in)

            kvp_f32 = xferp.tile([CQK + 1, C], F32, tag="kvp_f32")
            nc.sync.dma_start(kvp_f32, cc_out[:, 0:C])
            kvp = xferp.tile([CQK + 1, C], MDT, tag="kvp")
            nc.vector.tensor_copy(kvp, kvp_f32)
            ksum = xferp.tile([P, CQK + 1], F32, tag="ksum")
            nc.sync.dma_start(ksum[:, 0:CQK],
                              cc_out[0:CQK, C:C + 1].partition_broadcast(P))
            nc.vector.tensor_scalar_add(ksum[:, 0:CQK], ksum[:, 0:CQK], EPS)
            nc.vector.memset(ksum[:, CQK:CQK + 1], float(n_total))
            return kvp, ksum

        def ph2_gen(stash, kvp, ksum, first):
            """Phase-2 for one iteration, one macro-chunk per yield.

            Pipeline inside: chain(m) | mms(m-2) | drains(m-3).
            """
            def chain(m):
                st_sl = stash[:, m * ST:(m + 1) * ST, 0:CQK + 1]
                prod = scr2.tile([P, ST, CQK + 1], F32, tag="prod")
                nc.gpsimd.tensor_tensor(
                    prod, st_sl,
                    ksum[:, None, :].to_broadcast((P, ST, CQK + 1)), mult)
                den = scr2.tile([P, ST, 1], F32, tag="den")
                nc.vector.reduce_sum(den, prod, axis=mybir.AxisListType.X)
                d = scr2.tile([P, ST, 1], F32, tag="d")
                nc.vector.reciprocal(d, den)
                # rows padded to 64 so transposed PAIRS land on psum
                # partitions 0 and 64. col 33 picks up ||K||*d: finite junk.
                qsc = qscp.tile([P, ST, 2 * CQK], MDT, tag="qsc")
                if first and m < 4:  # first rotation: make pad finite
                    nc.gpsimd.memset(qsc[:, :, CQK + 2:2 * CQK], 0.0)
                nc.gpsimd.tensor_tensor(
                    qsc[:, :, 0:CQK + 2],
                    stash[:, m * ST:(m + 1) * ST, 0:CQK + 2],
                    d.to_broadcast((P, ST, CQK + 2)), mult)
                return qsc

            def mms(m, qsc):
                qt_ps = ps_qt.tile([P, ST // 2, P], MDT, tag="qt_ps")
                for j in range(ST // 2):
                    pair = qsc[:, 2 * j:2 * j + 2, :].rearrange(
                        "p a b -> p (a b)")  # [128, 128]
                    nc.tensor.transpose(qt_ps[:, j, :], pair, ident_r)
                qt_sb = qtp.tile([CQK + 1, ST, P], MDT, tag="qt_sb")
                nc.vector.tensor_copy(qt_sb[:, 0::2, :],
                                      qt_ps[0:CQK + 1, :, :])
                nc.vector.tensor_copy(qt_sb[:, 1::2, :],
                                      qt_ps[2 * CQK:2 * CQK + CQK + 1, :, :])
                pss = []
                for nh in range(2):
                    o_ps = ps_mm.tile([P, 2, HNT], PSDT, tag="mm2")
                    for blk in range(2):
                        nc.tensor.matmul(
                            o_ps[:, blk, :],
                            kvp[:, blk * P:(blk + 1) * P],
                            qt_sb[:, nh * (ST // 2):(nh + 1) * (ST // 2), :],
                            start=True,
                            stop=True,
                        )
                    pss.append(o_ps)
                return pss

            def drains(m, pss):
                ot = outp.tile([P, 2, NT], ODT, tag="ot")
                nc.vector.tensor_copy(ot[:, 0, 0:HNT], pss[0][:, 0, :])
                nc.scalar.copy(ot[:, 1, 0:HNT], pss[0][:, 1, :])
                nc.scalar.copy(ot[:, :, HNT:NT], pss[1])
                nc.sync.dma_start(out_r[:, :, m * NT:(m + 1) * NT], ot)

            chains = [(0, chain(0))]
            yield
            chains.append((1, chain(1)))
            yield
            mmed = []
            for m in range(2, NM + 3):
                if m < NM:
                    chains.append((m, chain(m)))
                if chains:
                    mm_m, qsc_mm = chains.pop(0)
                    mmed.append((mm_m, mms(mm_m, qsc_mm)))
                if len(mmed) == 2 or m >= NM + 1:
                    dr_m, pss_dr = mmed.pop(0)
                    drains(dr_m, pss_dr)
                yield

        # ---------------- pipelined repeat loop ----------------
        step = 0
        gen = None          # active phase-2 generator (prev iteration)
        gen_ready_at = 0
        for it in range(repeat):
            stash = stashp.tile([P, SROW, SW], MDT, tag="stash")
            kv_acc = ps_kv.tile([P, 512], F32, tag="kv_acc")
            xt_state = {"stash": stash, "prev": [], "kv_acc": kv_acc,
                        "kv_emitted": 0}
            for m in range(NM):
                ph1_macro(m, xt_state)
                step += 1
                if gen is not None and step >= gen_ready_at:
                    if next(gen, StopIteration) is StopIteration:
                        gen = None
            kvp, ksum = ph1_tail(xt_state)
            # flush the remainder of the previous iteration's phase 2 (runs
            # under this iteration's AllReduce latency)
            if gen is not None:
                for _ in gen:
                    pass
                gen = None
            if 2 in phases:
                gen = ph2_gen(stash, kvp, ksum, first=(it == 0))
                gen_ready_at = step + skew
            else:
                src = xs_r[:, :, 0:NT]
                if MDT != ODT:
                    src = src.bitcast(ODT)
                nc.sync.dma_start(out_r[:, :, 0:NT], src)
        if gen is not None:
            for _ in gen:
                pass

    nc.compile()
    return nc


_NC_CACHE = {}


def _get_nc(nsh, n_total, num_cores, groups_key, mm_dtype="bf16"):
    key = (nsh, n_total, num_cores, groups_key, mm_dtype)
    if key not in _NC_CACHE:
        groups = [list(g) for g in groups_key]
        _NC_CACHE[key] = build_attention_nc(nsh, n_total, num_cores, groups,
                                            mm_dtype)
    return _NC_CACHE[key]


def _kernel_numpy(x, wq, bq, wk, bk, wv, bv):
    """Plain numpy fallback (used only for nonzero biases / odd shapes)."""
    b, c, h, w = x.shape
    n = h * w
    xf = x.reshape(b, c, n).astype(np.float64)
    Q = np.einsum("oc,bcn->bon", wq.astype(np.float64), xf) + bq.astype(np.float64)[None, :, None]
    K = np.einsum("oc,bcn->bon", wk.astype(np.float64), xf) + bk.astype(np.float64)[None, :, None]
    V = np.einsum("oc,bcn->bon", wv.astype(np.float64), xf) + bv.astype(np.float64)[None, :, None]
    Qn = Q / np.linalg.norm(Q, axis=1, keepdims=True)
    Kn = K / np.linalg.norm(K, axis=1, keepdims=True)
    k_sum = Kn.sum(-1) + EPS
    tailor = 1.0 / (n + np.einsum("bmn,bm->bn", Qn, k_sum))
    value_sum = V.sum(-1)
    kv = np.einsum("bmn,bcn->bmc", Kn, V)
    ms = value_sum[:, :, None] + np.einsum("bmn,bmc->bcn", Qn, kv)
    return (ms * tailor[:, None, :]).reshape(b, c, h, w).astype(np.float32)


def kernel(x, wq, bq, wk, bk, wv, bv):
    x = np.asarray(x, dtype=np.float32)
    B, Cc, H, W = x.shape
    if (any(np.any(np.asarray(b_) != 0) for b_ in (bq, bk, bv))
            or Cc != C or wq.shape != (CQK, C) or wv.shape != (C, C)
            or (H * W) % (2 * NT) != 0 or B != 4):
        return _kernel_numpy(x, wq, bq, wk, bk, wv, bv)
    N = H * W
    ncores = 8
    shards_per_batch = ncores // B  # 2
    nsh = N // shards_per_batch  # 32768
    groups_key = tuple(
        tuple(range(b * shards_per_batch, (b + 1) * shards_per_batch))
        for b in range(B)
    )

    mm_dtype = "bf16"
    io_t = _np_io(mm_dtype)
    wt = np.ascontiguousarray(
        np.concatenate([np.asarray(wq).T, np.asarray(wk).T, np.asarray(wv).T],
                       axis=1).astype(io_t))

    nc = _get_nc(nsh, N, ncores, groups_key, mm_dtype)

    xr = x.reshape(B, Cc, N).astype(io_t)
    in_maps = []
    for core in range(ncores):
        b, hh = core // shards_per_batch, core % shards_per_batch
        in_maps.append({
            "xs": np.ascontiguousarray(xr[b, :, hh * nsh:(hh + 1) * nsh]),
            "wt": wt,
        })

    res = run_bass_kernel_spmd(nc, in_maps, list(range(ncores)))

    out = np.empty((B, Cc, N), np.float32)
    for core in range(ncores):
        b, hh = core // shards_per_batch, core % shards_per_batch
        out[b, :, hh * nsh:(hh + 1) * nsh] = np.asarray(
            res.results[core]["out"]).astype(np.float32)
    return out.reshape(B, Cc, H, W)
